# revision 13
# baseline (speedup 1.0000x reference)
"""DigitCapsules routing kernel for 8 Trainium2 NeuronCores.

Math: in the reference, u_hat is an explicit broadcast of u_core over the
capsule axis i, so b stays constant along i in every routing iteration,
softmax over i is exactly uniform (1/K), and the whole 3-iteration routing
collapses (exactly, in floating point too) to:

    v[b, i, :] = squash((1/576) * sum_{r,k} x2[b, r, k] * W[b, r, k, :])

broadcast over i = 0..575, where x2 = x.reshape(B, 8, 576).transpose(0, 2, 1).

Sharding: batch dim B=32 across 8 cores, 4 batches per core (data parallel,
per the hint).  Per core: contract over (r, k)=4608 on TensorE (x columns
stationary, W moving, fp32 PSUM accumulation, 5 r-tiles per batch), take the
k-diagonal of each per-batch [8, 128] result via a one-hot-mask multiply +
grouped DVE reduction (runs per batch so all but the last are hidden under
the next batch's matmuls / DMA wait), column-sum the four per-batch [8, 16]
partials into one [4, 16] PSUM tile with tiny fp32r one-hot matmuls, squash
all four batches in one pass, spread v across all 128 partitions with a
single fp32r select matmul, and store the i-broadcast output with ONE
[128 x 1152B] DMA that engages all 16 DMA engines.

Performance notes (vs the previous per-pair version: 21.2us -> this):
 - The measured exec window includes ~8.2us of fixed NEFF machinery (runtime
   preamble const-memsets anchor the window start; NRT injects ~253
   per-semaphore reset instructions before the final branch).  Only the
   ~13us body was optimizable; this version batches the extraction/squash/
   broadcast tail (one pass over [4,*] tiles instead of two over [2,*]) and
   replaces 2 output DMAs + 8 fp32 LOW/HIGH colsum matmuls with 1 DMA + 4
   single-pass fp32r matmuls.
 - The host packs wx = [W | x2] rows partition-major and pre-converts to
   fp16 ([NB, 128, 680]): halves HBM traffic and runs the matmuls at
   1 cycle/row instead of fp32's 4 (PSUM accumulation stays fp32; measured
   end-to-end relative error ~3e-4 against the fp32 reference).
 - All DMAs use flat 2D access patterns with >=1 KB per-partition runs.
 - Bacc (not raw Bass): its compile() splits sync waits into event
   semaphores (TRN2 allows one wait per instruction).
 - tensor_tensor_reduce (custom DVE op) hard-crashes the exec unit on this
   runtime - avoid.
"""

import numpy as np

import concourse.bacc as bacc
import concourse.mybir as mybir
import concourse.tile as tile
from concourse.bass_utils import run_bass_kernel_spmd

N_CORES = 8
B, C, H, W_ = 32, 8, 24, 24
R = H * W_          # 576 routes
RP = 640            # padded routes (5 tiles of 128)
KJ = 128            # fused (k=8, j=16) axis
D = 16
NB = B // N_CORES   # 4 batches per core
NTILE = RP // 128   # 5
WX = KJ + C         # 136 = W row + packed x2 row
FREE = NTILE * WX   # 680 fp16 values per partition
RNORM = 1.0 / float(R)
RNORM2 = RNORM * RNORM

_cached_nc = None
_last_in_maps = None


def _build():
    nc = bacc.Bacc(trn_type="TRN2")
    f32 = mybir.dt.float32
    f32r = mybir.dt.float32r
    f16 = mybir.dt.float16

    wx_h = nc.dram_tensor("wx", [NB // 2, 128, 2 * FREE], f16,
                          kind="ExternalInput")
    out_h = nc.dram_tensor("out", [NB, R, D], f32, kind="ExternalOutput")

    with tile.TileContext(nc) as tc:
        with (
            tc.tile_pool(name="consts", bufs=1) as consts,
            tc.tile_pool(name="wp", bufs=NB) as wp,
            tc.tile_pool(name="gps", bufs=NB, space="PSUM") as gps,
            tc.tile_pool(name="tps", bufs=1, space="PSUM") as tps,
            tc.tile_pool(name="vps", bufs=1, space="PSUM") as vps,
            tc.tile_pool(name="sm", bufs=16) as sm,
        ):
            # mask[k, j*8+k'] = (k == k'): selects the k-diagonal of G
            # (W columns are host-packed j-major so the grouped reduce sums
            # a contiguous innermost k' axis).
            mask_raw = consts.tile([8, KJ], f32)
            nc.gpsimd.memset(mask_raw[:], 1.0)
            nc.gpsimd.affine_select(
                out=mask_raw[:], in_=mask_raw[:],
                compare_op=mybir.AluOpType.is_equal, fill=0.0,
                base=0, pattern=[[0, 16], [1, 8]], channel_multiplier=-1,
            )
            # batch-local one-hot columns: matmul n uses oneh4[:, 4n:4n+4]
            # (all-ones in column n, zero elsewhere) so the accumulating
            # colsum matmuls route batch n's k-sum into row n of t_all.
            oneh_raw = consts.tile([8, 4 * NB], f32)
            nc.vector.memset(oneh_raw[:], 0.0)
            for n in range(NB):
                nc.vector.memset(oneh_raw[:, 5 * n:5 * n + 1], 1.0)
            oneh4 = consts.tile([8, 4 * NB], f32r)
            nc.vector.tensor_copy(oneh4[:], oneh_raw[:])
            eps_t = consts.tile([NB, 1], f32)
            nc.vector.memset(eps_t[:], 1e-8)
            # sel4[n, p] = (p // 32 == n): spreads v across all 128
            # partitions so the output DMA engages all 16 DMA engines
            # (engine = source partition mod 16).
            sel_raw = consts.tile([NB, 128], f32)
            nc.gpsimd.memset(sel_raw[:], 1.0)
            nc.gpsimd.affine_select(
                out=sel_raw[:], in_=sel_raw[:],
                compare_op=mybir.AluOpType.is_ge, fill=0.0,
                base=0, pattern=[[1, 128]], channel_multiplier=-32,
            )
            nc.gpsimd.affine_select(
                out=sel_raw[:], in_=sel_raw[:],
                compare_op=mybir.AluOpType.is_ge, fill=0.0,
                base=31, pattern=[[-1, 128]], channel_multiplier=32,
            )
            # fp16 so the broadcast matmul gets FWL weight loads and
            # 1-cycle/column streaming (v components are O(0.1); fp16
            # round-off ~5e-4 relative, well inside the 2e-2 budget).
            sel4 = consts.tile([NB, 128], f16)
            nc.vector.tensor_copy(sel4[:], sel_raw[:])

            # Per-batch contraction G[n][k, j*8+k'] = sum_r x2[n,r,k] *
            # W[n,r,j*8+k'], then the k-diagonal partial sums
            # r1_all[k, n*16+j] = sum_k' G[n][k, j*8+k'] * (k==k').
            # Each batch's mask-mul + grouped reduce runs as soon as its 5
            # matmuls stop, hidden under the next batch's DMA/matmuls.
            r1_all = sm.tile([8, NB * D], f32r)
            for pr in range(NB // 2):
                wx_t = wp.tile([128, 2 * FREE], f16)
                # One DMA per batch pair: 2.7 KB per-partition runs halve
                # the descriptor count and lift per-engine streaming to
                # line rate; a single dma_start stripes its partition rows
                # over all 16 DMA engines.
                eng = nc.sync if pr % 2 == 0 else nc.scalar
                eng.dma_start(wx_t[:], wx_h[pr])
                wx_v = wx_t[:].rearrange("p (d f) -> p d f", f=WX)
                for nl in range(2):
                    n = 2 * pr + nl
                    g_b = gps.tile([8, KJ], f32, tag="g_b")
                    for d in range(NTILE):
                        nc.tensor.matmul(
                            g_b[:],
                            wx_v[:, nl * NTILE + d, KJ:WX],
                            wx_v[:, nl * NTILE + d, :KJ],
                            start=(d == 0), stop=(d == NTILE - 1),
                        )
                    pm = sm.tile([8, KJ], f32, tag="pm")
                    nc.vector.tensor_mul(pm[:], g_b[:], mask_raw[:])
                    # f32r out has the same bits as f32 — only tagged so
                    # the colsum matmuls run single-pass, not LOW/HIGH.
                    with nc.allow_low_precision("f32r == f32 bitwise"):
                        nc.vector.reduce_sum(
                            r1_all[:, n * D:(n + 1) * D],
                            pm[:].rearrange("p (j k) -> p j k", j=D),
                            axis=mybir.AxisListType.X,
                        )

            # T[n, j] = sum_k r1_all[k, n*16+j]: four tiny accumulating
            # fp32r matmuls; the one-hot stationary routes batch n into
            # row n of t_all (matmul out base partition must be 0).
            t_all = tps.tile([NB, D], f32)
            for n in range(NB):
                nc.tensor.matmul(
                    t_all[:], oneh4[:, 4 * n:4 * n + 4],
                    r1_all[:, n * D:(n + 1) * D],
                    start=(n == 0), stop=(n == NB - 1),
                )

            # Batched squash over all 4 batches:
            #   normT = sum_j T^2;  norm = normT/576^2
            #   v = T * (norm/576) / ((1+norm) * sqrt(norm + 1e-8))
            # (square on DVE: scalar.square would evict Sqrt's ACT table)
            t_sb = sm.tile([NB, D], f32)
            nc.vector.tensor_copy(t_sb[:], t_all[:])
            sq = sm.tile([NB, D], f32)
            nc.vector.tensor_mul(sq[:], t_sb[:], t_sb[:])
            norm_t = sm.tile([NB, 1], f32)
            nc.vector.reduce_sum(norm_t[:], sq[:], axis=mybir.AxisListType.X)
            q = sm.tile([NB, 1], f32)
            nc.scalar.activation(
                q[:], norm_t[:], mybir.ActivationFunctionType.Sqrt,
                bias=eps_t[:], scale=RNORM2,
            )
            a1 = sm.tile([NB, 1], f32)
            nc.vector.tensor_scalar(
                out=a1[:], in0=norm_t[:], scalar1=RNORM2, scalar2=1.0,
                op0=mybir.AluOpType.mult, op1=mybir.AluOpType.add,
            )
            den = sm.tile([NB, 1], f32)
            nc.vector.tensor_mul(den[:], a1[:], q[:])
            rec = sm.tile([NB, 1], f32)
            nc.vector.reciprocal(rec[:], den[:])
            c1 = sm.tile([NB, 1], f32)
            nc.vector.tensor_scalar_mul(c1[:], norm_t[:], RNORM2 * RNORM)
            v1 = sm.tile([NB, D], f16)
            nc.vector.tensor_scalar(
                out=v1[:], in0=t_all[:], scalar1=c1[:], scalar2=rec[:],
                op0=mybir.AluOpType.mult, op1=mybir.AluOpType.mult,
            )

            # Spread v across all 128 partitions (partition p gets batch
            # p//32's v, 18 copies) and store with ONE 147 KB DMA that
            # engages all 16 DMA engines.  The PSUM evict runs on ScalarE
            # (closer to PSUM: 383 ns vs DVE's 425 for [*, 288]).
            vb_ps = vps.tile([128, 18 * D], f32)
            nc.tensor.matmul(
                vb_ps[:], sel4[:],
                v1[:].unsqueeze(1).broadcast_to([NB, 18, D]),
                start=True, stop=True)
            vb = sm.tile([128, 18 * D], f32)
            nc.scalar.copy(vb[:], vb_ps[:])
            dst = out_h[:, :, :].flatten().rearrange(
                "(p c) -> p c", c=18 * D)
            nc.sync.dma_start(dst, vb[:])

    nc.finalize()
    return nc


def kernel(x, route_weights):
    global _cached_nc, _last_in_maps
    if _cached_nc is None:
        _cached_nc = _build()
    nc = _cached_nc

    x = np.ascontiguousarray(np.asarray(x), dtype=np.float32)
    w = np.ascontiguousarray(np.asarray(route_weights), dtype=np.float32)
    x2 = x.reshape(B, C, R).transpose(0, 2, 1)          # [B, R, 8]
    # j-major column packing: wf[b, r, j*8+k] = W[b, r, k, j]
    wf = w.reshape(B, R, C, D).transpose(0, 1, 3, 2).reshape(B, R, KJ)
    wx = np.zeros((B, RP, WX), np.float32)
    wx[:, :R, :KJ] = wf
    wx[:, :R, KJ:] = x2
    # partition-major tiling, fp16, pair-merged: [B/2, 128, 2*NTILE*WX]
    wxt = (wx.reshape(B, NTILE, 128, WX).transpose(0, 2, 1, 3)
           .reshape(B // 2, 2, 128, FREE).transpose(0, 2, 1, 3)
           .reshape(B // 2, 128, 2 * FREE)).astype(np.float16)

    npair = NB // 2
    in_maps = [
        {"wx": np.ascontiguousarray(wxt[c * npair:(c + 1) * npair])}
        for c in range(N_CORES)
    ]
    _last_in_maps = in_maps

    res = run_bass_kernel_spmd(nc, in_maps, core_ids=list(range(N_CORES)))
    return np.concatenate([r["out"] for r in res.results], axis=0)
